# revision 3
# baseline (speedup 1.0000x reference)
"""AttGRU cell on 8 TRN2 NeuronCores.

Math (per reference):
    agg = einsum('ij,bj->bi', adj, x)                  # [B, N]
    r   = sigmoid(agg + h @ W_hr.T + b_hr)
    z   = sigmoid(agg + h @ W_hz.T + b_hz)
    n   = tanh(agg + r * (h @ W_hn.T + b_hn))
    out = (1 - z) * n + z * h

B=8, N=4096. Memory-bound: the four [N, N] f32 matrices (256 MB) dominate.

Sharding: row-shard adj/W_* over 8 cores (512 output features per core),
replicate x/h (tiny). Each core computes its 512 output columns; the host
concatenates. No collectives.

Per-core layout (host-prepared):
  wall [11, 128, 6144] bf16 - the four sharded+transposed matrices packed as
       33 contraction chunks of [128, 2048] ( [adjT | WhrT | WhzT | WhnT]
       chunks, 512 cols each ), 3 chunks per DMA slab. Chunk 32 is a bias
       chunk: row 0 = [0 | b_hr | b_hz | b_hn], so biases ride the matmul.
  vt   [128, 528] bf16 - stationary operand: [x.T | h.T] per chunk
       ([128, 16] each), chunk 32 is [0 | ones-row] to activate the biases.
  hloc [8, 512] f32 - h column shard for the output blend.

Each chunk k: psum_g[16, 512] += vt_k.T @ wall_k_g for the 4 gates; rows 0:8
hold the x-products (used for adj), rows 8:16 the h-products (used for W_*).
bf16 halves HBM traffic vs f32 and streams at 1 cycle/row on the PE
(f32 is 4 cycles/row); accumulation stays f32 in PSUM. rel err ~3e-3.
"""

from contextlib import ExitStack

import ml_dtypes
import numpy as np

import concourse.bass as bass
import concourse.tile as tile
from concourse import bacc, mybir
from concourse.bass_utils import run_bass_kernel_spmd

B = 8
N = 4096
NCORES = 8
S = N // NCORES          # 512 output cols per core
KC = 128                 # contraction chunk (PE partition dim)
NK = N // KC             # 32 data chunks
NKB = NK + 1             # +1 bias chunk
GROUP = 3                # chunks per DMA slab (33 = 11 * 3)
NDMA = NKB // GROUP      # 11
GW = 4 * S               # 2048: four gates side by side
M2 = 2 * B               # 16: [x | h] stationary columns

BF16 = mybir.dt.bfloat16
F32 = mybir.dt.float32

_CACHED_NC = None


def _build():
    nc = bacc.Bacc(
        "TRN2",
        target_bir_lowering=False,
        debug=False,
        num_devices=NCORES,
    )
    wall = nc.dram_tensor("wall", [NDMA, KC, GROUP * GW], BF16, kind="ExternalInput")
    vt = nc.dram_tensor("vt", [KC, NKB * M2], BF16, kind="ExternalInput")
    hloc = nc.dram_tensor("hloc", [B, S], F32, kind="ExternalInput")
    out = nc.dram_tensor("out", [B, S], F32, kind="ExternalOutput")

    AF = mybir.ActivationFunctionType

    with tile.TileContext(nc) as tc, ExitStack() as ctx:
        wpool = ctx.enter_context(tc.tile_pool(name="wall", bufs=3))
        cpool = ctx.enter_context(tc.tile_pool(name="const", bufs=1))
        ppool = ctx.enter_context(tc.tile_pool(name="acc", bufs=1, space="PSUM"))
        epool = ctx.enter_context(tc.tile_pool(name="epi", bufs=1))

        vt_sb = cpool.tile([KC, NKB * M2], BF16, tag="vt")
        nc.sync.dma_start(vt_sb[:], vt[:])
        hloc_sb = cpool.tile([B, S], F32, tag="hloc")
        nc.sync.dma_start(hloc_sb[:], hloc[:])

        acc = [
            ppool.tile([B, S], F32, tag=f"acc{g}", name=f"acc{g}") for g in range(4)
        ]

        for i in range(NDMA):
            wt = wpool.tile([KC, GROUP * GW], BF16, tag="wt")
            nc.sync.dma_start(wt[:], wall[i])
            for c in range(GROUP):
                k = i * GROUP + c
                lhsT_x = vt_sb[:, k * M2 : k * M2 + B]
                lhsT_h = vt_sb[:, k * M2 + B : (k + 1) * M2]
                for g in range(4):
                    nc.tensor.matmul(
                        acc[g][:, :],
                        lhsT_x if g == 0 else lhsT_h,
                        wt[:, c * GW + g * S : c * GW + (g + 1) * S],
                        start=(k == 0),
                        stop=(k == NKB - 1),
                    )

        agg = acc[0][:, :]          # x @ adjT (+0 bias)
        mm_r = acc[1][:, :]         # h @ WhrT + b_hr
        mm_z = acc[2][:, :]
        mm_n = acc[3][:, :]

        s_agg = epool.tile([B, S], F32, tag="sagg")
        nc.scalar.copy(s_agg[:], agg)

        t_r = epool.tile([B, S], F32, tag="tr")
        nc.vector.tensor_add(t_r[:], mm_r, s_agg[:])
        r_t = epool.tile([B, S], F32, tag="r")
        nc.scalar.activation(r_t[:], t_r[:], AF.Sigmoid)

        t_z = epool.tile([B, S], F32, tag="tz")
        nc.vector.tensor_add(t_z[:], mm_z, s_agg[:])
        z_t = epool.tile([B, S], F32, tag="z")
        nc.scalar.activation(z_t[:], t_z[:], AF.Sigmoid)

        t_n = epool.tile([B, S], F32, tag="tn")
        nc.vector.tensor_mul(t_n[:], mm_n, r_t[:])
        t_n2 = epool.tile([B, S], F32, tag="tn2")
        nc.vector.tensor_add(t_n2[:], t_n[:], s_agg[:])
        n_t = epool.tile([B, S], F32, tag="n")
        nc.scalar.activation(n_t[:], t_n2[:], AF.Tanh)

        d_t = epool.tile([B, S], F32, tag="d")
        nc.vector.tensor_sub(d_t[:], hloc_sb[:], n_t[:])
        zd_t = epool.tile([B, S], F32, tag="zd")
        nc.vector.tensor_mul(zd_t[:], d_t[:], z_t[:])
        o_t = epool.tile([B, S], F32, tag="o")
        nc.vector.tensor_add(o_t[:], zd_t[:], n_t[:])

        nc.sync.dma_start(out[:], o_t[:])

    nc.compile()
    return nc


def _get_nc():
    global _CACHED_NC
    if _CACHED_NC is None:
        _CACHED_NC = _build()
    return _CACHED_NC


def make_in_maps(x, h, adj, W_hr, b_hr, W_hz, b_hz, W_hn, b_hn):
    bf = ml_dtypes.bfloat16
    x = np.asarray(x, np.float32)
    h = np.asarray(h, np.float32)
    adj = np.asarray(adj, np.float32)
    W_hr = np.asarray(W_hr, np.float32)
    W_hz = np.asarray(W_hz, np.float32)
    W_hn = np.asarray(W_hn, np.float32)
    b_hr = np.asarray(b_hr, np.float32)
    b_hz = np.asarray(b_hz, np.float32)
    b_hn = np.asarray(b_hn, np.float32)

    vt_full = np.zeros((NKB * KC, M2), np.float32)
    vt_full[:N, :B] = x.T
    vt_full[:N, B:] = h.T
    vt_full[N, B:] = 1.0  # bias-chunk ones row (h side only)
    vt_packed = np.ascontiguousarray(
        vt_full.reshape(NKB, KC, M2).transpose(1, 0, 2).reshape(KC, NKB * M2)
    ).astype(bf)

    in_maps = []
    for s in range(NCORES):
        rs, re = s * S, (s + 1) * S
        wallf = np.zeros((NKB * KC, GW), np.float32)
        wallf[:N, 0:S] = adj[rs:re].T
        wallf[:N, S : 2 * S] = W_hr[rs:re].T
        wallf[:N, 2 * S : 3 * S] = W_hz[rs:re].T
        wallf[:N, 3 * S :] = W_hn[rs:re].T
        wallf[N, S : 2 * S] = b_hr[rs:re]
        wallf[N, 2 * S : 3 * S] = b_hz[rs:re]
        wallf[N, 3 * S :] = b_hn[rs:re]
        wallp = np.ascontiguousarray(
            wallf.reshape(NDMA, GROUP, KC, GW)
            .transpose(0, 2, 1, 3)
            .reshape(NDMA, KC, GROUP * GW)
        ).astype(bf)
        in_maps.append(
            {
                "wall": wallp,
                "vt": vt_packed,
                "hloc": np.ascontiguousarray(h[:, rs:re]),
            }
        )
    return in_maps


def run(in_maps, trace=False, **kw):
    nc = _get_nc()
    return run_bass_kernel_spmd(
        nc, in_maps, core_ids=list(range(NCORES)), trace=trace, **kw
    )


def kernel(x, h, adj, W_hr, b_hr, W_hz, b_hz, W_hn, b_hn):
    in_maps = make_in_maps(x, h, adj, W_hr, b_hr, W_hz, b_hz, W_hn, b_hn)
    res = run(in_maps)
    return np.concatenate(
        [np.asarray(res.results[s]["out"]) for s in range(NCORES)], axis=1
    )


# revision 4
# speedup vs baseline: 1.0564x; 1.0564x over previous
"""AttGRU cell on 8 TRN2 NeuronCores.

Math (per reference):
    agg = einsum('ij,bj->bi', adj, x)                  # [B, N]
    r   = sigmoid(agg + h @ W_hr.T + b_hr)
    z   = sigmoid(agg + h @ W_hz.T + b_hz)
    n   = tanh(agg + r * (h @ W_hn.T + b_hn))
    out = (1 - z) * n + z * h

B=8, N=4096. Memory-bound: the four [N, N] f32 matrices (256 MB) dominate.

Sharding: row-shard adj/W_* over 8 cores (512 output features per core),
replicate x/h (tiny). Each core computes its 512 output columns; the host
concatenates. No collectives.

v2: gate-major weight streaming (adj -> W_hr -> W_hn -> W_hz) so each
gate's epilogue overlaps the next gate's DMA stream; only the z-gate tail
stays serial. tanh computed as 2*sigmoid(2u)-1 so ScalarE keeps a single
activation table (no mid-tail table reload).

Per-core inputs (host-prepared):
  wall [12, 128, 5632] bf16 - per gate (adj, Whr, Whn, Whz): the sharded,
       transposed matrix as 33 contraction chunks of [128, 512] (chunk 32
       is the bias row-chunk so biases ride the matmul), grouped 11 chunks
       per DMA slab -> 3 slabs per gate.
  vt   [128, 528] bf16 - stationary operand: [x.T | h.T] per chunk
       ([128, 16]); chunk 32 is [0 | ones-row] to activate the biases.
  hloc [8, 512] f32 - h column shard for the output blend.

bf16 halves HBM traffic vs f32 and streams at 1 cycle/row on the PE
(f32 is 4 cycles/row); accumulation stays f32 in PSUM. rel err ~1.3e-3.
"""

from contextlib import ExitStack

import ml_dtypes
import numpy as np

import concourse.bass as bass
import concourse.tile as tile
from concourse import bacc, mybir
from concourse.bass_utils import run_bass_kernel_spmd

B = 8
N = 4096
NCORES = 8
S = N // NCORES          # 512 output cols per core
KC = 128                 # contraction chunk (PE partition dim)
NK = N // KC             # 32 data chunks
NKB = NK + 1             # +1 bias chunk
CHUNKS_PER_SLAB = 11     # 33 = 3 * 11
SLABS_PER_GATE = NKB // CHUNKS_PER_SLAB  # 3
SLABW = CHUNKS_PER_SLAB * S              # 5632
M2 = 2 * B               # 16: [x | h] stationary columns

BF16 = mybir.dt.bfloat16
F32 = mybir.dt.float32

_CACHED_NC = None


def _build():
    nc = bacc.Bacc(
        "TRN2",
        target_bir_lowering=False,
        debug=False,
        num_devices=NCORES,
    )
    wall = nc.dram_tensor(
        "wall", [4 * SLABS_PER_GATE, KC, SLABW], BF16, kind="ExternalInput"
    )
    vt = nc.dram_tensor("vt", [KC, NKB * M2], BF16, kind="ExternalInput")
    hloc = nc.dram_tensor("hloc", [B, S], F32, kind="ExternalInput")
    out = nc.dram_tensor("out", [B, S], F32, kind="ExternalOutput")

    AF = mybir.ActivationFunctionType
    ALU = mybir.AluOpType

    with tile.TileContext(nc) as tc, ExitStack() as ctx:
        wpool = ctx.enter_context(tc.tile_pool(name="wall", bufs=3))
        cpool = ctx.enter_context(tc.tile_pool(name="const", bufs=1))
        ppool = ctx.enter_context(tc.tile_pool(name="acc", bufs=1, space="PSUM"))
        epool = ctx.enter_context(tc.tile_pool(name="epi", bufs=1))

        vt_sb = cpool.tile([KC, NKB * M2], BF16, tag="vt")
        nc.gpsimd.dma_start(vt_sb[:], vt[:])
        hloc_sb = cpool.tile([B, S], F32, tag="hloc")
        nc.gpsimd.dma_start(hloc_sb[:], hloc[:])

        acc = [
            ppool.tile([B, S], F32, tag=f"acc{g}", name=f"acc{g}") for g in range(4)
        ]

        # epilogue tiles, declared up front
        s_agg = epool.tile([B, S], F32, tag="sagg")
        t_r = epool.tile([B, S], F32, tag="tr")
        r_t = epool.tile([B, S], F32, tag="r")
        t_n = epool.tile([B, S], F32, tag="tn")
        t_n2 = epool.tile([B, S], F32, tag="tn2")
        sg_t = epool.tile([B, S], F32, tag="sg")
        n_t = epool.tile([B, S], F32, tag="n")
        d_t = epool.tile([B, S], F32, tag="d")
        t_z = epool.tile([B, S], F32, tag="tz")
        z_t = epool.tile([B, S], F32, tag="z")
        zd_t = epool.tile([B, S], F32, tag="zd")
        o_t = epool.tile([B, S], F32, tag="o")

        # gates in stream order: 0=adj, 1=W_hr, 2=W_hn, 3=W_hz
        for g in range(4):
            for sl in range(SLABS_PER_GATE):
                wt = wpool.tile([KC, SLABW], BF16, tag="wt", name=f"wt{g}_{sl}")
                nc.sync.dma_start(wt[:], wall[g * SLABS_PER_GATE + sl])
                for c in range(CHUNKS_PER_SLAB):
                    k = sl * CHUNKS_PER_SLAB + c
                    if g == 0:
                        lhsT = vt_sb[:, k * M2 : k * M2 + B]          # x part
                    else:
                        lhsT = vt_sb[:, k * M2 + B : (k + 1) * M2]    # h part
                    nc.tensor.matmul(
                        acc[g][:, :],
                        lhsT,
                        wt[:, c * S : (c + 1) * S],
                        start=(k == 0),
                        stop=(k == NKB - 1),
                    )
            # per-gate epilogue; Tile starts each as soon as deps clear
            if g == 0:
                nc.vector.tensor_copy(s_agg[:], acc[0][:, :])
            elif g == 1:
                nc.vector.tensor_add(t_r[:], acc[1][:, :], s_agg[:])
                nc.scalar.activation(r_t[:], t_r[:], AF.Sigmoid)
            elif g == 2:
                nc.vector.tensor_mul(t_n[:], acc[2][:, :], r_t[:])
                nc.vector.tensor_add(t_n2[:], t_n[:], s_agg[:])
                # tanh(u) = 2*sigmoid(2u) - 1 (keeps ACT on one table)
                nc.scalar.activation(sg_t[:], t_n2[:], AF.Sigmoid, scale=2.0)
                nc.vector.tensor_scalar(
                    n_t[:], sg_t[:], 2.0, 1.0, ALU.mult, ALU.subtract
                )
                nc.vector.tensor_sub(d_t[:], hloc_sb[:], n_t[:])
            else:
                nc.vector.tensor_add(t_z[:], acc[3][:, :], s_agg[:])
                nc.scalar.activation(z_t[:], t_z[:], AF.Sigmoid)
                nc.vector.tensor_mul(zd_t[:], z_t[:], d_t[:])
                nc.vector.tensor_add(o_t[:], zd_t[:], n_t[:])

        nc.sync.dma_start(out[:], o_t[:])

    nc.compile()
    return nc


def _get_nc():
    global _CACHED_NC
    if _CACHED_NC is None:
        _CACHED_NC = _build()
    return _CACHED_NC


def make_in_maps(x, h, adj, W_hr, b_hr, W_hz, b_hz, W_hn, b_hn):
    bf = ml_dtypes.bfloat16
    x = np.asarray(x, np.float32)
    h = np.asarray(h, np.float32)
    adj = np.asarray(adj, np.float32)
    W_hr = np.asarray(W_hr, np.float32)
    W_hz = np.asarray(W_hz, np.float32)
    W_hn = np.asarray(W_hn, np.float32)
    b_hr = np.asarray(b_hr, np.float32)
    b_hz = np.asarray(b_hz, np.float32)
    b_hn = np.asarray(b_hn, np.float32)

    vt_full = np.zeros((NKB * KC, M2), np.float32)
    vt_full[:N, :B] = x.T
    vt_full[:N, B:] = h.T
    vt_full[N, B:] = 1.0  # bias-chunk ones row (h side only)
    vt_packed = np.ascontiguousarray(
        vt_full.reshape(NKB, KC, M2).transpose(1, 0, 2).reshape(KC, NKB * M2)
    ).astype(bf)

    in_maps = []
    for s in range(NCORES):
        rs, re = s * S, (s + 1) * S
        # stream order: adj, W_hr, W_hn, W_hz (z last -> shortest tail)
        slabs = []
        for W, b in (
            (adj, None),
            (W_hr, b_hr),
            (W_hn, b_hn),
            (W_hz, b_hz),
        ):
            gm = np.zeros((NKB * KC, S), np.float32)
            gm[:N] = W[rs:re].T
            if b is not None:
                gm[N] = b[rs:re]
            slabs.append(
                gm.reshape(SLABS_PER_GATE, CHUNKS_PER_SLAB, KC, S)
                .transpose(0, 2, 1, 3)
                .reshape(SLABS_PER_GATE, KC, SLABW)
            )
        wallp = np.ascontiguousarray(np.concatenate(slabs, axis=0)).astype(bf)
        in_maps.append(
            {
                "wall": wallp,
                "vt": vt_packed,
                "hloc": np.ascontiguousarray(h[:, rs:re]),
            }
        )
    return in_maps


def run(in_maps, trace=False, **kw):
    nc = _get_nc()
    return run_bass_kernel_spmd(
        nc, in_maps, core_ids=list(range(NCORES)), trace=trace, **kw
    )


def kernel(x, h, adj, W_hr, b_hr, W_hz, b_hz, W_hn, b_hn):
    in_maps = make_in_maps(x, h, adj, W_hr, b_hr, W_hz, b_hz, W_hn, b_hn)
    res = run(in_maps)
    return np.concatenate(
        [np.asarray(res.results[s]["out"]) for s in range(NCORES)], axis=1
    )
